# revision 18
# baseline (speedup 1.0000x reference)
"""Trainium2 Bass kernel for nn_MemResProjections (memory-residual attention).

Reference computation (B=4, S=2048, K=64, H=1024, fp32):
    normed = rmsnorm(hidden) * norm_w
    v_pool = concat([normed, memory], axis=1)            # (B, S+K, H)
    q = normed @ q_w.T ; k = v_pool @ k_w.T
    logits = q @ k.T / sqrt(H)  with causal mask on the local S block,
    memory columns fully visible
    attn = softmax(logits); h~ = attn @ v_pool
    alpha = sigmoid(hidden @ gate_w.T + gate_b)
    out = (1-alpha)*hidden + alpha*h~

Key algebraic restructure: k and q are never materialized.
    scores = q @ k.T = nrm @ W2 @ [nrm | mem/w].T,  W2 = (q_w w)^T (k_w w)
with nrm = x * rstd (no norm_w); W2 is precomputed on the host (H x H).
The norm_w factor is folded out of v:  h~ = (attn @ [nrm | mem/w]) * w.
Per-core device work: q'' = nrm @ W2, gate, scores, h~ — 4.8 GMAC in bf16.

Sharding: 8 cores = (batch b, parity h).  Core (b,h) owns query stripes
s = h, h+2, ..., h+14 (128 rows each) — interleaving balances the causal
triangle.  The host stages x rows owned-stripes-first, making the SPMD
program uniform: for slot k the score loop visits positions 0..k (own
parity, diag tri mask at p==k) and 8..8+k (other parity, where position
8+k is fully-masked on h=0 / fully-visible on h=1 via a per-core bias
column), plus the 64 memory rows.

All operands live in SBUF in bf16; HBM traffic is x (bf16, once), the
H^2 weights (bf16), and the fp32 output.  DMAs are split into <=128KB
chunks across many queues (per-queue DMA sustains only ~25GB/s) and
dispatched from both sync and scalar engines.
"""
import numpy as np

import concourse.bass as bass
import concourse.mybir as mybir
import concourse.tile as tile
from concourse.bass_utils import run_bass_kernel_spmd

F32 = mybir.dt.float32
BF16 = mybir.dt.bfloat16
AFT = mybir.ActivationFunctionType
ALU = mybir.AluOpType

P = 128
H = 1024
NJ = H // P           # h blocks
NS = 8                # owned query stripes (slots) per core
NPOS = 16             # sequence stripes per batch
T_MEM = 64
SCALE = 1.0 / 32.0    # 1/sqrt(H)
EPS = 1e-6
NEG = -1.0e30

N_CORES = 8
B_FULL, S_FULL = 4, 2048


# ---------------------------------------------------------------- walrus fix
ENGINE_ATTR = {
    mybir.EngineType.PE: "tensor",
    mybir.EngineType.Activation: "scalar",
    mybir.EngineType.DVE: "vector",
    mybir.EngineType.Pool: "gpsimd",
    mybir.EngineType.SP: "sync",
}
DMA_OPS = ("InstDMACopy", "InstDMATranspose", "InstTensorLoad", "InstTensorSave",
           "InstCollectiveCompute")


def split_multi_waits(nc, limit=1, dma_limit=None):
    """This walrus build rejects engine instructions carrying more than one
    sem wait; hoist extras onto same-engine NOPs inserted just before."""
    n_split = 0
    for f in nc.m.functions:
        for blk in f.blocks:
            il = blk.instructions
            i = 0
            while i < len(il):
                ins = il[i]
                is_dma = type(ins).__name__ in DMA_OPS
                lim = dma_limit if is_dma else limit
                si = ins.sync_info
                waits = list(si.on_wait) if si is not None and si.on_wait else []
                if lim is not None and len(waits) > lim:
                    keep, extra = waits[:lim], waits[lim:]
                    si.on_wait.clear()
                    for w in keep:
                        si.on_wait.append(w)
                    eng = getattr(nc, ENGINE_ATTR[ins.engine])
                    for w in extra:
                        nop = eng.nop(nofuse=True, hint="wait_split")
                        nop.wait_op(bass.SemaphoreHandle(w.ant_name, w.id),
                                    w.wait_value, "sem-ge")
                        popped = nc.cur_bb.bb.instructions.pop()
                        assert popped.name == nop.ins.name
                        il.insert(i, nop.ins)
                        i += 1
                        n_split += 1
                i += 1
    return n_split


# ---------------------------------------------------------------- program
def build_nc():
    nc = bass.Bass()
    dpb = lambda n, shp: nc.declare_dram_parameter(n, shp, BF16, isOutput=False)
    xbf = dpb("xbf", [NPOS * P, H])    # x rows, owned-stripes-first, bf16
    w2 = dpb("w2", [H, H])             # (q_w*w).T @ (k_w*w)
    gwT = dpb("gwT", [H, H])           # gate_w.T
    memT2 = dpb("memT2", [H, T_MEM])   # (mem / norm_w).T
    mem2 = dpb("mem2", [T_MEM, H])     # mem / norm_w
    ident = dpb("ident", [P, P])
    dpf = lambda n, shp: nc.declare_dram_parameter(n, shp, F32, isOutput=False)
    w_bc = dpf("w_bc", [P, H])     # norm_w broadcast
    b_bc = dpf("b_bc", [P, H])     # gate_b broadcast
    pbias = dpf("pbias", [P, 1])   # -1e30 (h=0) or 0 (h=1)
    tri = dpf("tri", [P, P])       # additive mask: 0 if col>=row else -1e30
    out = nc.declare_dram_parameter("out", [NS * P, H], F32, isOutput=True)

    with tile.TileContext(nc) as tc:
        from contextlib import ExitStack
        with ExitStack() as ctx:
            # ---- constants (tiny DMAs first on sync)
            const = ctx.enter_context(tc.tile_pool(name="const", bufs=1))
            id_t = const.tile([P, P], BF16)
            nc.sync.dma_start(out=id_t[:], in_=ident[:])
            pb_t = const.tile([P, 1], F32)
            nc.sync.dma_start(out=pb_t[:], in_=pbias[:])
            tri_t = const.tile([P, P], F32)
            nc.sync.dma_start(out=tri_t[:], in_=tri[:])
            w_bc_t = const.tile([P, H], F32)
            eps_t = const.tile([P, 1], F32)
            nc.vector.memset(eps_t[:], EPS)
            ones_b = const.tile([P, 1], BF16)
            nc.vector.memset(ones_b[:], 1.0)
            std_all = const.tile([P, NPOS], F32)
            rstd_all = const.tile([P, NPOS], F32)
            rden = const.tile([P, NS], F32)

            b_bc_t = const.tile([P, H], F32)

            # memory tiles dispatched from gpsimd later (needed only at B1)
            memT_t = [const.tile([P, T_MEM], BF16, tag=f"mT{m}", name=f"mT{m}")
                      for m in range(NJ)]
            vmem = const.tile([T_MEM, H], BF16)

            vres = ctx.enter_context(tc.tile_pool(name="vres", bufs=1))
            v_nat = [vres.tile([P, H], BF16, tag=f"v{i}", name=f"v{i}")
                     for i in range(NPOS)]

            proj = ctx.enter_context(tc.tile_pool(name="proj", bufs=1))
            qsT = [proj.tile([P, NS * P], BF16, tag=f"qsT{m}", name=f"qsT{m}")
                   for m in range(NJ)]
            alpha = [proj.tile([P, H], BF16, tag=f"al{i}", name=f"al{i}")
                     for i in range(NS)]
            alphaw = [proj.tile([P, H], BF16, tag=f"aw{i}", name=f"aw{i}")
                      for i in range(NS)]

            ntp = ctx.enter_context(tc.tile_pool(name="ntp", bufs=1))
            # [partition, stripe, j*128+elem]
            nT = ntp.tile([P, NPOS, H], BF16)

            xst = ctx.enter_context(tc.tile_pool(name="xst", bufs=6))
            x_t = [xst.tile([P, H], BF16, tag="xt", name=f"xt{i}")
                   for i in range(NPOS)]
            wres_cm = tc.tile_pool(name="wres", bufs=1)
            wres = wres_cm.__enter__()
            w2_s = [wres.tile([P, H], BF16, tag=f"w2{j}", name=f"w2{j}")
                    for j in range(NJ)]
            gw_s = [wres.tile([P, H], BF16, tag=f"gw{j}", name=f"gw{j}")
                    for j in range(NJ)]
            # sync: x stripes 0-7 then gate weights
            for i in range(NS):
                for c in range(2):
                    nc.sync.dma_start(
                        out=x_t[i][c * 64:(c + 1) * 64, :],
                        in_=xbf[i * P + c * 64:i * P + (c + 1) * 64, :])
            for j in range(NJ):
                nc.sync.dma_start(out=gw_s[j][:],
                                  in_=gwT[j * P:(j + 1) * P, :])
            # gpsimd software DGE: w2 halves first, then gate bias /
            # norm_w bcast, then x stripes 8-15, then memory tiles
            for j in range(NJ):
                for c in range(2):
                    nc.gpsimd.dma_start(
                        out=w2_s[j][c * 64:(c + 1) * 64, :],
                        in_=w2[j * P + c * 64:j * P + (c + 1) * 64, :])
            for c in range(2):
                nc.gpsimd.dma_start(
                    out=b_bc_t[c * 64:(c + 1) * 64, :],
                    in_=b_bc[c * 64:(c + 1) * 64, :])
            for c in range(2):
                nc.gpsimd.dma_start(
                    out=w_bc_t[c * 64:(c + 1) * 64, :],
                    in_=w_bc[c * 64:(c + 1) * 64, :])
            for i in range(NS, NPOS):
                for c in range(2):
                    nc.gpsimd.dma_start(
                        out=x_t[i][c * 64:(c + 1) * 64, :],
                        in_=xbf[i * P + c * 64:i * P + (c + 1) * 64, :])

            with tc.tile_pool(name="warm", bufs=2, space="PSUM") as wps:
                for _ in range(40):
                    wt = wps.tile([P, P], BF16, tag="wt")
                    nc.tensor.transpose(wt[:], id_t[:], id_t[:])

            def a1_stripe(idx, ast, aps):
                sq = ast.tile([P, H], BF16, tag="sq")
                ss = ast.tile([P, 1], F32, tag="ss")
                nc.scalar.activation(sq[:], x_t[idx][:], AFT.Square,
                                     accum_out=ss[:])
                nc.scalar.activation(std_all[:, idx:idx + 1], ss[:],
                                     AFT.Sqrt, scale=1.0 / H, bias=eps_t[:])
                nc.vector.reciprocal(rstd_all[:, idx:idx + 1],
                                     std_all[:, idx:idx + 1])
                nc.vector.tensor_scalar_mul(v_nat[idx][:], x_t[idx][:],
                                            rstd_all[:, idx:idx + 1])
                for c in range(2):
                    tp = aps.tile([P, 512], BF16, tag="tp")
                    for j4 in range(4):
                        j = c * 4 + j4
                        nc.tensor.transpose(
                            tp[:, j4 * P:(j4 + 1) * P],
                            v_nat[idx][:, j * P:(j + 1) * P], id_t[:])
                    if c == 0:
                        nc.vector.tensor_copy(
                            nT[:, idx, c * 512:(c + 1) * 512], tp[:])
                    else:
                        nc.scalar.activation(
                            nT[:, idx, c * 512:(c + 1) * 512], tp[:],
                            AFT.Copy)

            with tc.tile_pool(name="a1s", bufs=3) as ast, \
                 tc.tile_pool(name="a1ps", bufs=4, space="PSUM") as aps:
                # ---- A1a: owned stripes
                for idx in range(NS):
                    a1_stripe(idx, ast, aps)

                # gate weights via scalar-engine queue (between A1a/A1b work)
                for j in range(NJ):
                    nc.scalar.dma_start(out=gw_s[j][:],
                                        in_=gwT[j * P:(j + 1) * P, :])
                for c in range(2):
                    nc.scalar.dma_start(
                        out=b_bc_t[c * 64:(c + 1) * 64, :],
                        in_=b_bc[c * 64:(c + 1) * 64, :])

                # ---- A2: q''T = W2.T @ nrm.T over owned stripes
                with tc.tile_pool(name="a2ps", bufs=3, space="PSUM") as ps2:
                    for sc in range(2):
                        for m in range(NJ):
                            pq = ps2.tile([P, 512], F32, tag="pq", name="pq")
                            for j in range(NJ):
                                nc.tensor.matmul(
                                    pq[:],
                                    w2_s[j][:, m * P:(m + 1) * P],
                                    nT[:, sc * 4:(sc + 1) * 4,
                                       j * P:(j + 1) * P],
                                    start=(j == 0), stop=(j == NJ - 1))
                            nc.vector.tensor_copy(
                                qsT[m][:, sc * 512:(sc + 1) * 512], pq[:])

                # ---- A1b: other-parity stripes
                for idx in range(NS, NPOS):
                    a1_stripe(idx, ast, aps)

                # memory tiles via gpsimd software DGE (needed at B1)
                for m in range(NJ):
                    nc.gpsimd.dma_start(out=memT_t[m][:],
                                        in_=memT2[m * P:(m + 1) * P, :])
                nc.gpsimd.dma_start(out=vmem[:], in_=mem2[:])

            # ---- A4: gate -> alpha (gate = (nrm @ gwT) * std)
            with tc.tile_pool(name="a4s", bufs=2) as gst, \
                 tc.tile_pool(name="a4ps", bufs=3, space="PSUM") as gps:
                for si in range(NS):
                    pg = [gps.tile([P, 512], F32, tag=f"pg{oc}",
                                   name=f"pg{oc}") for oc in range(2)]
                    for j in range(NJ):
                        for oc in range(2):
                            nc.tensor.matmul(
                                pg[oc][:],
                                nT[:, si, j * P:(j + 1) * P],
                                gw_s[j][:, oc * 512:(oc + 1) * 512],
                                start=(j == 0), stop=(j == NJ - 1))
                    for oc in range(2):
                        gl = gst.tile([P, 512], F32, tag="gl")
                        nc.vector.scalar_tensor_tensor(
                            gl[:], pg[oc][:], std_all[:, si:si + 1],
                            b_bc_t[:, oc * 512:(oc + 1) * 512],
                            ALU.mult, ALU.add)
                        nc.scalar.activation(
                            alpha[si][:, oc * 512:(oc + 1) * 512], gl[:],
                            AFT.Sigmoid)
                        nc.vector.tensor_mul(
                            alphaw[si][:, oc * 512:(oc + 1) * 512],
                            alpha[si][:, oc * 512:(oc + 1) * 512],
                            w_bc_t[:, oc * 512:(oc + 1) * 512])

            wres_cm.__exit__(None, None, None)

            etp = ctx.enter_context(tc.tile_pool(name="etp", bufs=1))
            et_e = [etp.tile([P, (NS - p) * P], BF16, tag=f"ete{p}",
                             name=f"ete{p}") for p in range(NS)]
            et_o = [etp.tile([P, (NS - p) * P], BF16, tag=f"eto{p}",
                             name=f"eto{p}") for p in range(NS)]
            et_m = etp.tile([T_MEM, NS * P], BF16)

            # ---- B1: scores^T -> exp tiles (SBUF, bf16)
            with tc.tile_pool(name="b1ps", bufs=4, space="PSUM") as bps:
                # memory rows first (unblocks every slot in B2)
                for c in range(2):
                    pm = bps.tile([P, 512], F32, tag="ps", name="pm")
                    for m in range(NJ):
                        nc.tensor.matmul(
                            pm[:T_MEM, :], memT_t[m][:],
                            qsT[m][:, c * 512:(c + 1) * 512],
                            start=(m == 0), stop=(m == NJ - 1))
                    nc.scalar.activation(
                        et_m[:, c * 512:(c + 1) * 512], pm[:T_MEM, :],
                        AFT.Exp, scale=SCALE)
                # interleave parities so B2 slot k unblocks early
                for p in range(NS):
                    w = (NS - p) * P
                    for half in range(2):   # 0 = own parity, 1 = other
                        pos = p + half * NS
                        nch = (w + 511) // 512
                        pc = []
                        for c in range(nch):
                            c0, c1 = c * 512, min(w, (c + 1) * 512)
                            psx = bps.tile([P, 512], F32, tag="ps",
                                           name="psx")
                            pc.append((psx, c0, c1))
                        for m in range(NJ):
                            for (psx, c0, c1) in pc:
                                nc.tensor.matmul(
                                    psx[:, :c1 - c0],
                                    nT[:, pos, m * P:(m + 1) * P],
                                    qsT[m][:, p * P + c0:p * P + c1],
                                    start=(m == 0), stop=(m == NJ - 1),
                                    skip_group_check=True)
                        et = et_e[p] if half == 0 else et_o[p]
                        for c, (psx, c0, c1) in enumerate(pc):
                            if c == 0:
                                if half == 0:
                                    # diagonal block: causal tri mask
                                    nc.vector.tensor_add(
                                        psx[:, 0:P], psx[:, 0:P], tri_t[:])
                                    nc.scalar.activation(
                                        et[:, c0:c1], psx[:, :c1 - c0],
                                        AFT.Exp, scale=SCALE)
                                else:
                                    # other-parity same-index stripe:
                                    # fully masked (h=0) / visible (h=1)
                                    nc.scalar.activation(
                                        et[:, 0:P], psx[:, 0:P],
                                        AFT.Exp, scale=SCALE, bias=pb_t[:])
                                    if c1 > P:
                                        nc.scalar.activation(
                                            et[:, P:c1], psx[:, P:c1 - c0],
                                            AFT.Exp, scale=SCALE)
                            else:
                                nc.scalar.activation(
                                    et[:, c0:c1], psx[:, :c1 - c0],
                                    AFT.Exp, scale=SCALE)

            # ---- B2: h~ accumulation + fused combine
            with tc.tile_pool(name="b2s", bufs=2) as bst, \
                 tc.tile_pool(name="b2ps", bufs=2, space="PSUM") as bph:
                am7 = bst.tile([P, H], F32, tag="am7", bufs=1)
                nc.scalar.activation(am7[:], alpha[NS - 1][:], AFT.Copy,
                                     scale=-1.0, bias=1.0)
                x27 = bst.tile([P, H], F32, tag="x27", bufs=1)
                nc.vector.scalar_tensor_tensor(
                    x27[:], v_nat[NS - 1][:], std_all[:, NS - 1:NS],
                    am7[:], ALU.mult, ALU.mult)
                for k in range(NS):
                    ph = [bph.tile([P, 512], F32, tag=f"ph{hc}",
                                   name=f"ph{hc}") for hc in range(2)]
                    pd = bph.tile([P, 1], F32, tag="pd")
                    stat = []
                    for p in range(k + 1):
                        stat.append((et_e[p], (k - p) * P, v_nat[p], P))
                        stat.append((et_o[p], (k - p) * P, v_nat[NS + p], P))
                    stat.append((et_m, k * P, vmem, T_MEM))
                    for ti, (et_t, c0, vt, rows) in enumerate(stat):
                        first, last = ti == 0, ti == len(stat) - 1
                        for hc in range(2):
                            nc.tensor.matmul(
                                ph[hc][:], et_t[:rows, c0:c0 + P],
                                vt[:rows, hc * 512:(hc + 1) * 512],
                                start=first, stop=last,
                                skip_group_check=True)
                        nc.tensor.matmul(
                            pd[:], et_t[:rows, c0:c0 + P], ones_b[:rows, :],
                            start=first, stop=last, skip_group_check=True)
                    nc.vector.reciprocal(rden[:, k:k + 1], pd[:])
                    # out = (1-alpha)*x + (alpha*w) * (ph/den)
                    if k == NS - 1:
                        x2 = x27
                    else:
                        am = bst.tile([P, H], F32, tag="am", bufs=1)
                        nc.scalar.activation(am[:], alpha[k][:], AFT.Copy,
                                             scale=-1.0, bias=1.0)
                        x2 = bst.tile([P, H], F32, tag="x2", bufs=1)
                        nc.vector.scalar_tensor_tensor(
                            x2[:], v_nat[k][:], std_all[:, k:k + 1], am[:],
                            ALU.mult, ALU.mult)
                    hsb = bst.tile([P, H], F32, tag="hsb")
                    nchunk = 4 if k == NS - 1 else 1
                    rs = P // nchunk
                    for hc in range(2):
                        h0 = hc * 512
                        nc.vector.scalar_tensor_tensor(
                            hsb[:, h0:h0 + 512], ph[hc][:],
                            rden[:, k:k + 1], alphaw[k][:, h0:h0 + 512],
                            ALU.mult, ALU.mult)
                        nc.vector.tensor_add(hsb[:, h0:h0 + 512],
                                             hsb[:, h0:h0 + 512],
                                             x2[:, h0:h0 + 512])
                        for c in range(nchunk):
                            nc.sync.dma_start(
                                out=out[k * P + c * rs:k * P + (c + 1) * rs,
                                        h0:h0 + 512],
                                in_=hsb[c * rs:(c + 1) * rs, h0:h0 + 512])

    import os
    if os.environ.get("NO_WAIT_SPLIT") != "1":
        split_multi_waits(nc, limit=1, dma_limit=1)
    return nc


_NC_CACHE = None
_LAST_IN_MAPS = None


def _get_nc():
    global _NC_CACHE
    if _NC_CACHE is None:
        _NC_CACHE = build_nc()
    return _NC_CACHE


def prepare_in_maps(hidden_states, memory_state, q_w, k_w, norm_w, gate_w,
                    gate_b):
    import ml_dtypes
    bf = ml_dtypes.bfloat16
    hidden_states = np.asarray(hidden_states, dtype=np.float32)
    memory_state = np.asarray(memory_state, dtype=np.float32)
    q_w = np.asarray(q_w, dtype=np.float32)
    k_w = np.asarray(k_w, dtype=np.float32)
    norm_w = np.asarray(norm_w, dtype=np.float32)
    gate_w = np.asarray(gate_w, dtype=np.float32)
    gate_b = np.asarray(gate_b, dtype=np.float32)

    qwT = (q_w * norm_w[None, :]).T
    kw2 = k_w * norm_w[None, :]
    w2 = np.ascontiguousarray(qwT @ kw2).astype(bf)
    gwT = np.ascontiguousarray(gate_w.T).astype(bf)
    w_bc = np.ascontiguousarray(np.broadcast_to(norm_w, (P, H)))
    b_bc = np.ascontiguousarray(np.broadcast_to(gate_b, (P, H)))
    tri = np.where(np.arange(P)[None, :] >= np.arange(P)[:, None],
                   np.float32(0.0), np.float32(NEG)).astype(np.float32)
    ident = np.eye(P, dtype=np.float32).astype(bf)
    wsafe = np.where(np.abs(norm_w) > 1e-8, norm_w, 1.0)

    xr = hidden_states.reshape(B_FULL, NPOS, P, H)
    in_maps = []
    for c in range(N_CORES):
        b, h = divmod(c, 2)
        perm = list(range(h, NPOS, 2)) + list(range(1 - h, NPOS, 2))
        xbf = np.ascontiguousarray(
            xr[b][perm].reshape(NPOS * P, H)).astype(bf)
        mem2 = np.ascontiguousarray(memory_state[b] / wsafe[None, :]).astype(bf)
        in_maps.append({
            "xbf": xbf,
            "w2": w2, "gwT": gwT,
            "memT2": np.ascontiguousarray(mem2.T),
            "mem2": mem2,
            "w_bc": w_bc, "b_bc": b_bc,
            "pbias": np.full((P, 1), NEG if h == 0 else 0.0, np.float32),
            "tri": tri, "ident": ident,
        })
    return in_maps


def kernel(**inputs):
    in_maps = prepare_in_maps(**inputs)
    global _LAST_IN_MAPS
    _LAST_IN_MAPS = in_maps
    nc = _get_nc()
    res = run_bass_kernel_spmd(nc, in_maps, list(range(N_CORES)))
    out = np.empty((B_FULL, S_FULL, H), dtype=np.float32)
    for c in range(N_CORES):
        b, h = divmod(c, 2)
        o = res.results[c]["out"].reshape(NS, P, H)
        for k in range(NS):
            s = 2 * k + h
            out[b, s * P:(s + 1) * P] = o[k]
    return out
